# revision 1
# baseline (speedup 1.0000x reference)
"""Multi-head attention with interleaved RoPE on 8 Trainium2 NeuronCores.

Strategy: tensor-parallel over heads. Each core owns 2 of the 16 heads:
  - Q/K/V weights column-sliced (256 cols/core), out_proj row-sliced.
  - Each core computes its heads' attention and a partial out-projection;
    the host sums the 8 partials (plus the bias term bv@Wo + bo folded
    out of the device program entirely -- softmax rows sum to 1, so the
    v-bias contributes exactly bv@Wo to every output row).

Device dataflow (per core; operand storage fp16 by default, fp32 psum):
  xT = transpose(x) via PE             [D, tok]
  qT/kT = Wq.T @ xT (+bias, RoPE)      [d_head, tok] per head
  v = xT.T @ Wv                        [tok, dv]
  expT_i = exp((kT_i.T @ qT)/128)      [keys, q]  (1/d fold via ACT scale;
                                       no max-subtract: |logit/d| < 1 here)
  outT = sum_i v_i.T @ expT_i          [dv, q]
  s = sum_i ones.T @ expT_i            [1, q] -> recip -> PE-broadcast
  ahatT = outT * recip(s)              [dv, q]
  partial = ahatT.T @ Wo_rows          [tok, D] -> DMA out (fp32)
"""

import os

import numpy as np

B = 2
N = 2048  # tokens per batch
D = 2048  # model dim
H = 16
HD = 128  # head dim
NCORES = 8
HPC = H // NCORES  # heads per core = 2
DLOC = HPC * HD  # local width = 256
DC = D // 128  # contraction chunks = 16
TQ = 512  # token-quarter size for the x-transpose staging buffer
NT = N // 128  # token tiles per batch = 16

# matmul operand dtype: "float16" (1 cyc/row, ~1e-3 rel err)
# or "float32" (4 cyc/row, exact)
MM_DT_NAME = os.environ.get("ATTN_MM_DT", "float16")

_COMPILED = {}


def _build_nc():
    import concourse.bacc as bacc
    import concourse.mybir as mybir
    import concourse.tile as tile

    f32 = mybir.dt.float32
    sd = getattr(mybir.dt, MM_DT_NAME)  # matmul operand storage dtype

    nc = bacc.Bacc("TRN2", target_bir_lowering=False, debug=False,
                   num_devices=NCORES)

    x_in = nc.dram_tensor("x", [B, DC, 128, N], sd, kind="ExternalInput").ap()
    wq_in = nc.dram_tensor("wq", [D, DLOC], sd, kind="ExternalInput").ap()
    wk_in = nc.dram_tensor("wk", [D, DLOC], sd, kind="ExternalInput").ap()
    wv_in = nc.dram_tensor("wv", [D, DLOC], sd, kind="ExternalInput").ap()
    wo_in = nc.dram_tensor("wo", [DLOC, D], sd, kind="ExternalInput").ap()
    bq_in = nc.dram_tensor("bq", [HPC, 128, 1], f32, kind="ExternalInput").ap()
    bk_in = nc.dram_tensor("bk", [HPC, 128, 1], f32, kind="ExternalInput").ap()
    cos_in = nc.dram_tensor("cosT", [HD, N], sd, kind="ExternalInput").ap()
    s2_in = nc.dram_tensor("s2T", [HD, N], sd, kind="ExternalInput").ap()
    out_p = nc.dram_tensor("out_p", [B, N, D], sd, kind="ExternalOutput").ap()

    Exp = mybir.ActivationFunctionType.Exp
    Ident = mybir.ActivationFunctionType.Identity
    inv_d = 1.0 / HD  # folds the module's two 1/sqrt(d) logit scalings

    # spread DMAs across engine queues (each engine issues on its own queue)
    _eng = [nc.sync, nc.scalar]
    _ectr = [0]

    def dma(out, in_):
        e = _eng[_ectr[0] % len(_eng)]
        _ectr[0] += 1
        e.dma_start(out=out, in_=in_)

    with tile.TileContext(nc) as tc:
        with (
            tc.tile_pool(name="persist", bufs=1) as pers,
            tc.tile_pool(name="ps", bufs=6, space="PSUM") as ps_pool,
            tc.tile_pool(name="pso", bufs=2, space="PSUM") as pso_pool,
            tc.tile_pool(name="pexp", bufs=6) as pexp_pool,
            tc.tile_pool(name="prope", bufs=4) as prope_pool,
            tc.tile_pool(name="pout", bufs=3) as pout_pool,
            tc.tile_pool(name="psml", bufs=3) as psml_pool,
            tc.tile_pool(name="prec", bufs=2) as prec_pool,
        ):
            # ---- persistent SBUF tensors ---------------------------------
            xT = pers.tile([128, DC, N], sd, tag="xT")
            wq_sb = pers.tile([128, DC, DLOC], sd, tag="wq_sb")
            wq_r = wq_in.rearrange("(a p) o -> p a o", p=128)
            for c in range(4):
                dma(wq_sb[:, 4 * c : 4 * c + 4], wq_r[:, 4 * c : 4 * c + 4])
            for dq in range(8):
                dma(xT[:, dq * 2 : (dq + 1) * 2, :],
                    x_in[0, dq * 2 : (dq + 1) * 2].rearrange("a p t -> p a t"))
            ones_col = pers.tile([128, 32], sd, tag="ones_col")
            nc.vector.memset(ones_col, 1.0)
            # dummy matmuls: warm the PE clock (HAM) while input DMAs land
            warm = pers.tile([128, 128], sd, tag="warm")
            nc.vector.memset(warm, 0.0)
            for _ in range(36):
                pw = ps_pool.tile([128, 128], f32, tag="pl", bufs=3)
                nc.tensor.matmul(pw, warm, warm, start=True, stop=True)
            ones_rows = pers.tile([64, 128], sd, tag="ones_rows")
            nc.vector.memset(ones_rows, 1.0)
            zb = pers.tile([128, 1], f32, tag="zb")
            nc.vector.memset(zb, 0.0)

            wk_sb = pers.tile([128, DC, DLOC], sd, tag="wk_sb")
            wv_sb = pers.tile([128, DC, DLOC], sd, tag="wv_sb")
            wk_r = wk_in.rearrange("(a p) o -> p a o", p=128)
            for c in range(4):
                dma(wk_sb[:, 4 * c : 4 * c + 4], wk_r[:, 4 * c : 4 * c + 4])
            wv_r = wv_in.rearrange("(a p) o -> p a o", p=128)
            for c in range(4):
                dma(wv_sb[:, 4 * c : 4 * c + 4], wv_r[:, 4 * c : 4 * c + 4])
            wo_sb = pers.tile([128, HPC, D], sd, tag="wo_sb")
            cos_sb = pers.tile([HD, N], sd, tag="cos_sb")
            s2_sb = pers.tile([HD, N], sd, tag="s2_sb")
            dma(cos_sb, cos_in)
            dma(s2_sb, s2_in)
            wo_r = wo_in.rearrange("(h p) d -> p h d", p=128)
            dma(wo_sb[:, 0:1], wo_r[:, 0:1])
            dma(wo_sb[:, 1:2], wo_r[:, 1:2])
            bq_sb = pers.tile([128, HPC], f32, tag="bq_sb")
            bk_sb = pers.tile([128, HPC], f32, tag="bk_sb")
            for h in range(HPC):
                nc.sync.dma_start(out=bq_sb[:, h : h + 1], in_=bq_in[h])
                nc.sync.dma_start(out=bk_sb[:, h : h + 1], in_=bk_in[h])

            qT = pers.tile([128, HPC, N], sd, tag="qT")
            kT = pers.tile([128, HPC, N], sd, tag="kT")
            v_sb = pers.tile([128, NT, DLOC], sd, tag="v_sb")
            ahat = pers.tile([128, HPC, N], sd, tag="ahat")
            s_store = pers.tile([64, N], f32, tag="s_store")
            r_f32 = pers.tile([64, N], f32, tag="r_f32")
            r16 = pers.tile([64, N], sd, tag="r16")

            # swap even/odd partitions within each 32-lane quadrant (RoPE)
            swap_mask = [i + 1 if i % 2 == 0 else i - 1 for i in range(32)]

            for b in range(B):
                # ======== load pre-transposed x for this batch ============
                nc.enter_named_scope(f"xload{b}", False)
                if b > 0:
                    for dq in range(8):
                        dma(xT[:, dq * 2 : (dq + 1) * 2, :],
                            x_in[b, dq * 2 : (dq + 1) * 2].rearrange(
                                "a p t -> p a t"))
                nc.leave_named_scope(f"xload{b}", None, False)
                # ======== projections =====================================
                nc.enter_named_scope(f"proj{b}", False)
                for wsb, bsb, dst in ((wq_sb, bq_sb, qT), (wk_sb, bk_sb, kT)):
                    for h in range(HPC):
                        for nch in range(N // 512):
                            pq = ps_pool.tile([128, 512], f32, tag="pl", bufs=3)
                            for dc in range(DC):
                                nc.tensor.matmul(
                                    pq,
                                    wsb[:, dc, h * 128 : (h + 1) * 128],
                                    xT[:, dc, nch * 512 : (nch + 1) * 512],
                                    start=(dc == 0),
                                    stop=(dc == DC - 1),
                                )
                            nc.vector.tensor_scalar_add(
                                dst[:, h, nch * 512 : (nch + 1) * 512], pq,
                                bsb[:, h : h + 1],
                            )
                for tt in range(NT):
                    pv = ps_pool.tile([128, DLOC], f32, tag="pl", bufs=3)
                    for dc in range(DC):
                        nc.tensor.matmul(
                            pv,
                            xT[:, dc, tt * 128 : (tt + 1) * 128],
                            wv_sb[:, dc, :],
                            start=(dc == 0),
                            stop=(dc == DC - 1),
                        )
                    nc.vector.tensor_copy(v_sb[:, tt, :], pv)

                nc.leave_named_scope(f"proj{b}", None, False)
                # ======== RoPE on qT/kT (in place, 512-wide chunks) ========
                nc.enter_named_scope(f"rope{b}", False)
                for dst in (qT, kT):
                    for h in range(HPC):
                        for c0 in range(0, N, 512):
                            src = dst[:, h, c0 : c0 + 512]
                            sw = prope_pool.tile([128, 512], sd, tag="sw")
                            tm = prope_pool.tile([128, 512], sd, tag="tm")
                            nc.vector.stream_shuffle(sw, src, swap_mask)
                            nc.vector.tensor_mul(tm, src, cos_sb[:, c0 : c0 + 512])
                            nc.vector.tensor_mul(sw, sw, s2_sb[:, c0 : c0 + 512])
                            nc.vector.tensor_add(src, tm, sw)

                nc.leave_named_scope(f"rope{b}", None, False)
                # ======== attention + out-projection, per 512-q-chunk ======
                nc.enter_named_scope(f"attn{b}", False)
                for j in range(N // 512):
                    jq = slice(j * 512, (j + 1) * 512)
                    po = [ps_pool.tile([128, 512], f32, tag="po", bufs=2,
                                       name=f"po{h}") for h in range(HPC)]
                    ps2 = ps_pool.tile([64, 512], f32, tag="ps2", bufs=1,
                                       name="ps2")
                    for i in range(NT):
                        for h in range(HPC):
                            pl = ps_pool.tile([128, 512], f32, tag="pl", bufs=3)
                            nc.tensor.matmul(
                                pl,
                                kT[:, h, i * 128 : (i + 1) * 128],
                                qT[:, h, jq],
                                start=True, stop=True,
                            )
                            ex = pexp_pool.tile([128, 512], sd, tag="ex")
                            nc.scalar.activation(ex, pl, Exp, bias=zb,
                                                 scale=inv_d)
                            nc.tensor.matmul(
                                po[h],
                                v_sb[:, i, h * 128 : (h + 1) * 128],
                                ex,
                                start=(i == 0), stop=(i == NT - 1),
                            )
                            nc.tensor.matmul(
                                ps2[32 * h : 32 * h + 32, :],
                                ones_col,
                                ex,
                                start=(i == 0), stop=(i == NT - 1),
                            )
                    # per-j tail: stash sums + unnormalized attn (frees psum),
                    # then normalize in the background of the next j's i-loop
                    nc.vector.tensor_copy(s_store[:, jq], ps2)
                    for h in range(HPC):
                        nc.vector.tensor_copy(ahat[:, h, jq], po[h])
                    nc.vector.reciprocal_approx_fast(r_f32[:, jq],
                                                     s_store[:, jq])
                    nc.vector.tensor_copy(r16[:, jq], r_f32[:, jq])
                    for h in range(HPC):
                        pb = ps_pool.tile([128, 512], f32, tag="pl", bufs=3)
                        nc.tensor.matmul(
                            pb,
                            ones_rows[32 * h : 32 * h + 1, :],
                            r16[32 * h : 32 * h + 1, jq],
                            start=True, stop=True,
                        )
                        nc.vector.tensor_mul(ahat[:, h, jq],
                                             ahat[:, h, jq], pb)
                # out-projection for the whole batch
                for tt in range(NT):
                    trow = slice(tt * 128, (tt + 1) * 128)
                    for n in range(D // 512):
                        pp = pso_pool.tile([128, 512], f32, tag="pso")
                        for h in range(HPC):
                            nc.tensor.matmul(
                                pp,
                                ahat[:, h, tt * 128 : (tt + 1) * 128],
                                wo_sb[:, h, n * 512 : (n + 1) * 512],
                                start=(h == 0), stop=(h == HPC - 1),
                            )
                        ob = pout_pool.tile([128, 512], sd, tag="ob")
                        if n % 2 == 0:
                            nc.vector.tensor_copy(ob, pp)
                        else:
                            nc.scalar.copy(ob, pp)
                        oe = nc.sync if n % 2 == 0 else nc.scalar
                        oe.dma_start(
                            out=out_p[b, trow, n * 512 : (n + 1) * 512],
                            in_=ob)
                nc.leave_named_scope(f"attn{b}", 0, False)
    nc.compile()
    return nc


def _get_nc():
    if "nc" not in _COMPILED:
        _COMPILED["nc"] = _build_nc()
    return _COMPILED["nc"]


def _rope_tables():
    inv = (1.0 / (np.float32(10000.0)
                  ** (np.arange(0, HD, 2, dtype=np.float32) / np.float32(HD))))
    inv = inv.astype(np.float32)
    t = np.arange(N, dtype=np.float32)
    freqs = t[:, None] * inv[None, :]  # [N, HD/2]
    cosT = np.repeat(np.cos(freqs).astype(np.float32).T, 2, axis=0)  # [HD, N]
    s2T = np.repeat(np.sin(freqs).astype(np.float32).T, 2, axis=0)
    s2T = s2T.copy()
    s2T[0::2, :] *= np.float32(-1.0)
    return np.ascontiguousarray(cosT), np.ascontiguousarray(s2T)


def _make_in_maps(x, Wq, bq, Wk, bk, Wv, Wo):
    sd = np.float16 if MM_DT_NAME == "float16" else np.float32
    cosT, s2T = _rope_tables()
    cosT = cosT.astype(sd)
    s2T = s2T.astype(sd)
    # pre-transpose x on the host: [B, N, D] -> [B, DC, 128, N]
    xt = np.ascontiguousarray(
        np.asarray(x).transpose(0, 2, 1).reshape(B, DC, 128, N).astype(sd))
    in_maps = []
    for c in range(NCORES):
        cols = slice(c * DLOC, (c + 1) * DLOC)
        in_maps.append({
            "x": xt,
            "wq": np.ascontiguousarray(Wq[:, cols]).astype(sd),
            "wk": np.ascontiguousarray(Wk[:, cols]).astype(sd),
            "wv": np.ascontiguousarray(Wv[:, cols]).astype(sd),
            "wo": np.ascontiguousarray(Wo[cols, :]).astype(sd),
            "bq": np.ascontiguousarray(bq[cols].reshape(HPC, 128, 1)
                                       .astype(np.float32)),
            "bk": np.ascontiguousarray(bk[cols].reshape(HPC, 128, 1)
                                       .astype(np.float32)),
            "cosT": cosT,
            "s2T": s2T,
        })
    return in_maps


def run_device(x, Wq, bq, Wk, bk, Wv, bv, Wo, bo, trace=False):
    """Run the 8-core kernel; returns (full_output, BassKernelResults)."""
    from concourse.bass_utils import run_bass_kernel_spmd

    nc = _get_nc()
    in_maps = _make_in_maps(x, Wq, bq, Wk, bk, Wv, Wo)
    res = run_bass_kernel_spmd(nc, in_maps, core_ids=list(range(NCORES)),
                               trace=trace)
    acc = np.zeros((B, N, D), dtype=np.float64)
    for c in range(NCORES):
        acc += res.results[c]["out_p"]
    bias = (bv.astype(np.float64) @ Wo.astype(np.float64)
            + bo.astype(np.float64))
    out = (acc + bias).astype(np.float32)
    return out, res


def kernel(x, Wq, bq, Wk, bk, Wv, bv, Wo, bo):
    out, _ = run_device(x, Wq, bq, Wk, bk, Wv, bv, Wo, bo, trace=False)
    return out



# revision 2
# speedup vs baseline: 1.1565x; 1.1565x over previous
"""Multi-head attention with interleaved RoPE on 8 Trainium2 NeuronCores.

Tensor-parallel over heads (2 heads/core), restructured for continuous PE
occupancy (TRN2 PE drops to 1.2 GHz for 3us after any idle gap):

  - Q/K projections in fp8 e4m3 DoubleRow (K=256 per matmul, 2x PE rate).
    Only q/k are quantized; their error enters softmax logits where it is
    attenuated by the 1/d scaling, so the end-to-end error stays ~1e-2
    of absmax. V projection / attention / out-proj stay fp16.
  - Attention is software-pipelined (AV lags logits by 2 steps) so the PE
    never waits on the Scalar-engine Exp.
  - Projections for batch b+1 and the out-projection for batch b-1 are
    interleaved into the attention PE stream as fillers; the PE stream
    never has a dependency stall.
  - Softmax normalization is applied via a partition-broadcast of 1/s and
    one fused psum->sbuf multiply per (j, head).
  - Host folds softmax(.)@bv contribution (bv@Wo + bo) out of the device
    program and sums the 8 partial out-projections.

Layouts (per core):
  x8   [128, 8, 2, N] fp8   d-major, k-pair packed for DoubleRow
  xcol [NT][128, DC, 128]   tok-major fp16 columns for the V projection
  qT/kT[b] [128, HPC, N]    fp16, RoPE applied in place
  v[b] [128, NT, DLOC] fp16 (tok on partitions)
  ex   [128, 1024] fp16     exp(logits/d) for both heads of one key chunk
"""

import numpy as np

B = 2
N = 2048
D = 2048
H = 16
HD = 128
NCORES = 8
HPC = H // NCORES      # heads per core = 2
DLOC = HPC * HD        # local width = 256
DC = D // 128          # contraction chunks = 16
NT = N // 128          # token tiles = 16
NJ = N // 512          # 512-wide q blocks = 4

_COMPILED = {}


def _build_nc():
    import concourse.bacc as bacc
    import concourse.mybir as mybir
    import concourse.tile as tile

    f32 = mybir.dt.float32
    f16 = mybir.dt.float16
    f8 = mybir.dt.float8e4
    DR = mybir.MatmulPerfMode.DoubleRow
    Exp = mybir.ActivationFunctionType.Exp
    Ident = mybir.ActivationFunctionType.Identity
    inv_d = 1.0 / HD  # folds the module's two 1/sqrt(d) logit scalings

    nc = bacc.Bacc("TRN2", target_bir_lowering=False, debug=False,
                   num_devices=NCORES)

    x8_in = nc.dram_tensor("x8", [B, 128, 8, 2, N], f8,
                           kind="ExternalInput").ap()
    xc_in = nc.dram_tensor("xc", [B, NT, 128, DC, 128], f16,
                           kind="ExternalInput").ap()
    wq8_in = nc.dram_tensor("wq8", [128, 8, 2, DLOC], f8,
                            kind="ExternalInput").ap()
    wk8_in = nc.dram_tensor("wk8", [128, 8, 2, DLOC], f8,
                            kind="ExternalInput").ap()
    wv_in = nc.dram_tensor("wv", [128, DC, DLOC], f16,
                           kind="ExternalInput").ap()
    wo_in = nc.dram_tensor("wo", [128, HPC, D], f16,
                           kind="ExternalInput").ap()
    bq_in = nc.dram_tensor("bq", [128, HPC], f32, kind="ExternalInput").ap()
    bk_in = nc.dram_tensor("bk", [128, HPC], f32, kind="ExternalInput").ap()
    cos_in = nc.dram_tensor("cosT", [HD, N], f16, kind="ExternalInput").ap()
    s2_in = nc.dram_tensor("s2T", [HD, N], f16, kind="ExternalInput").ap()
    out_p = nc.dram_tensor("out_p", [B, N, D], f16, kind="ExternalOutput").ap()

    # swap even/odd partitions within each 32-lane quadrant (RoPE rotate)
    swap_mask = [i + 1 if i % 2 == 0 else i - 1 for i in range(32)]

    with tile.TileContext(nc) as tc:
        with (
            tc.tile_pool(name="persist", bufs=1) as pers,
            tc.tile_pool(name="pl", bufs=2, space="PSUM") as pl_pool,
            tc.tile_pool(name="po", bufs=1, space="PSUM") as po_pool,
            tc.tile_pool(name="ps2", bufs=1, space="PSUM") as ps2_pool,
            tc.tile_pool(name="pj", bufs=1, space="PSUM") as pj_pool,
            tc.tile_pool(name="pex", bufs=4) as ex_pool,
            tc.tile_pool(name="prp", bufs=8) as rope_pool,
            tc.tile_pool(name="pxc", bufs=4) as xcol_pool,
            tc.tile_pool(name="pob", bufs=4) as ob_pool,
            tc.tile_pool(name="prr", bufs=4) as r_pool,
            tc.tile_pool(name="prb", bufs=3) as rbc_pool,
        ):
            # ---- persistent SBUF tensors ---------------------------------
            wk8_sb = pers.tile([128, 8, 2, DLOC], f8, tag="wk8_sb")
            wq8_sb = pers.tile([128, 8, 2, DLOC], f8, tag="wq8_sb")
            x8_sb = pers.tile([128, 8, 2, N], f8, tag="x8_sb")
            wv_sb = pers.tile([128, DC, DLOC], f16, tag="wv_sb")
            wo_sb = pers.tile([128, HPC, D], f16, tag="wo_sb")
            cos_sb = pers.tile([HD, N], f16, tag="cos_sb")
            s2_sb = pers.tile([HD, N], f16, tag="s2_sb")
            bq_sb = pers.tile([128, HPC], f32, tag="bq_sb")
            bk_sb = pers.tile([128, HPC], f32, tag="bk_sb")
            zb = pers.tile([128, 1], f32, tag="zb")
            ones1 = pers.tile([128, 1], f16, tag="ones1")
            warm = pers.tile([128, 128], f16, tag="warm")

            qT = [pers.tile([128, HPC, N], f16, tag=f"qT{b}", name=f"qT{b}")
                  for b in range(B)]
            kT = [pers.tile([128, HPC, N], f16, tag=f"kT{b}", name=f"kT{b}")
                  for b in range(B)]
            v_sb = [pers.tile([128, NT, DLOC], f16, tag=f"v{b}", name=f"v{b}")
                    for b in range(B)]
            ahat = [pers.tile([128, HPC, N], f16, tag=f"ah{b}", name=f"ah{b}")
                    for b in range(B)]

            # initial loads: sync ring carries weights + x8(b0); gpsimd ring
            # carries the small tables.
            nc.sync.dma_start(out=wk8_sb, in_=wk8_in)
            nc.sync.dma_start(out=wq8_sb, in_=wq8_in)
            for c in range(4):
                nc.sync.dma_start(out=x8_sb[:, 2 * c : 2 * c + 2],
                                  in_=x8_in[0, :, 2 * c : 2 * c + 2])
            nc.sync.dma_start(out=wv_sb, in_=wv_in)
            nc.sync.dma_start(out=wo_sb, in_=wo_in)
            nc.gpsimd.dma_start(out=cos_sb, in_=cos_in)
            nc.gpsimd.dma_start(out=s2_sb, in_=s2_in)
            nc.gpsimd.dma_start(out=bq_sb, in_=bq_in)
            nc.gpsimd.dma_start(out=bk_sb, in_=bk_in)

            nc.vector.memset(zb, 0.0)
            nc.vector.memset(ones1, 1.0)
            nc.vector.memset(warm, 0.0)

            # warm the PE p-state while the input DMAs land
            for _ in range(60):
                pw = pj_pool.tile([128, 128], f32, tag="pj", name="pw")
                nc.tensor.matmul(pw, warm, warm, start=True, stop=True)

            # ---- filler generators --------------------------------------
            _cctr = [0]  # alternator for psum->sbuf copies
            _dctr = [0]  # alternator for output DMA rings

            def qk_chains(b):
                """fp8 DoubleRow Q/K projections + fused bias/RoPE epilogue.
                Yields one closure per PE matmul."""
                for w8sb, bcol, dst in ((wk8_sb, bk_sb, kT[b]),
                                        (wq8_sb, bq_sb, qT[b])):
                    for h in range(HPC):
                        for nch in range(NJ):
                            nsl = slice(nch * 512, (nch + 1) * 512)
                            pq = pj_pool.tile([128, 512], f32, tag="pj",
                                              name="pq")

                            def mk(c, pq=pq, w8sb=w8sb, h=h, nsl=nsl,
                                   bcol=bcol, dst=dst):
                                def emit():
                                    nc.tensor.matmul(
                                        pq,
                                        w8sb[:, c, :, h * 128 : (h + 1) * 128],
                                        x8_sb[:, c, :, nsl],
                                        start=(c == 0), stop=(c == 7),
                                        perf_mode=DR,
                                    )
                                    if c == 7:
                                        q1 = rope_pool.tile([128, 512], f16,
                                                            tag="rp", name="q1")
                                        nc.scalar.activation(
                                            q1, pq, Ident,
                                            bias=bcol[:, h : h + 1], scale=1.0)
                                        sw0 = rope_pool.tile([128, 512], f16,
                                                             tag="rp",
                                                             name="sw0")
                                        nc.vector.stream_shuffle(sw0, q1,
                                                                 swap_mask)
                                        tm = rope_pool.tile([128, 512], f16,
                                                            tag="rp", name="tm")
                                        nc.vector.tensor_mul(
                                            tm, q1, cos_sb[:, nsl])
                                        sw1 = rope_pool.tile([128, 512], f16,
                                                             tag="rp",
                                                             name="sw1")
                                        nc.gpsimd.tensor_mul(
                                            sw1, sw0, s2_sb[:, nsl])
                                        nc.vector.tensor_add(
                                            dst[:, h, nsl], tm, sw1)
                                return emit

                            for c in range(8):
                                yield mk(c)

            def v_chains(b):
                """fp16 V projection from streamed token-column tiles."""
                for tt in range(NT):
                    xc = xcol_pool.tile([128, DC, 128], f16, tag="xc",
                                        name="xc")
                    pv = pj_pool.tile([128, DLOC], f32, tag="pj", name="pv")

                    def mk(dc, xc=xc, pv=pv, tt=tt, b=b):
                        def emit():
                            if dc == 0:
                                nc.sync.dma_start(out=xc, in_=xc_in[b, tt])
                            nc.tensor.matmul(
                                pv, xc[:, dc, :], wv_sb[:, dc, :],
                                start=(dc == 0), stop=(dc == DC - 1))
                            if dc == DC - 1:
                                nc.scalar.copy(v_sb[b][:, tt, :], pv)
                        return emit

                    for dc in range(DC):
                        yield mk(dc)

            def o_chains(b):
                """fp16 out-projection of normalized attention output."""
                for tt in range(NT):
                    tsl = slice(tt * 128, (tt + 1) * 128)
                    for n in range(4):
                        nsl = slice(n * 512, (n + 1) * 512)
                        pp = pj_pool.tile([128, 512], f32, tag="pj", name="pp")

                        def mk(h, pp=pp, tsl=tsl, nsl=nsl, b=b):
                            def emit():
                                nc.tensor.matmul(
                                    pp, ahat[b][:, h, tsl],
                                    wo_sb[:, h, nsl],
                                    start=(h == 0), stop=(h == HPC - 1))
                                if h == HPC - 1:
                                    ob = ob_pool.tile([128, 512], f16,
                                                      tag="ob", name="ob")
                                    if _cctr[0] % 2 == 0:
                                        nc.scalar.copy(ob, pp)
                                    else:
                                        nc.vector.tensor_copy(ob, pp)
                                    _cctr[0] += 1
                                    eng = (nc.sync if _dctr[0] % 2 == 0
                                           else nc.gpsimd)
                                    _dctr[0] += 1
                                    eng.dma_start(out=out_p[b, tsl, nsl],
                                                  in_=ob)
                            return emit

                        for h in range(HPC):
                            yield mk(h)

            def run_all(gen):
                for emit in gen:
                    emit()

            # ---- attention with pipelined drain + fillers ----------------
            def attention(b, fillers, n_fill):
                """64 steps of (2 logits mm, Exp, lagged 2 AV + 2 sum mms),
                popping fillers to keep the PE stream dense."""
                popped = [0]
                step = [0]

                def pop_fillers():
                    step[0] += 1
                    want = (n_fill * step[0]) // (NJ * NT)
                    while popped[0] < want:
                        emit = next(fillers, None)
                        if emit is None:
                            popped[0] = n_fill
                            return
                        emit()
                        popped[0] += 1

                for j in range(NJ):
                    jq = slice(j * 512, (j + 1) * 512)
                    po = [po_pool.tile([128, 512], f32, tag=f"po{h}",
                                       name=f"po{h}") for h in range(HPC)]
                    ps2 = ps2_pool.tile([64, 512], f32, tag="ps2", name="ps2")
                    exq = []

                    def drain_one():
                        i2, ex2 = exq.pop(0)
                        for h in range(HPC):
                            exh = ex2[:, h * 512 : (h + 1) * 512]
                            nc.tensor.matmul(
                                po[h], v_sb[b][:, i2, h * 128 : (h + 1) * 128],
                                exh, start=(i2 == 0), stop=(i2 == NT - 1))
                            nc.tensor.matmul(
                                ps2[32 * h : 32 * h + 1, :], ones1, exh,
                                start=(i2 == 0), stop=(i2 == NT - 1))

                    for i in range(NT):
                        pl = pl_pool.tile([128, 1024], f32, tag="pl",
                                          name="pl")
                        for h in range(HPC):
                            nc.tensor.matmul(
                                pl[:, h * 512 : (h + 1) * 512],
                                kT[b][:, h, i * 128 : (i + 1) * 128],
                                qT[b][:, h, jq],
                                start=True, stop=True)
                        ex = ex_pool.tile([128, 1024], f16, tag="ex",
                                          name="ex")
                        nc.scalar.activation(ex, pl, Exp, bias=zb,
                                             scale=inv_d)
                        exq.append((i, ex))
                        if len(exq) > 2:
                            drain_one()
                        pop_fillers()
                    while exq:
                        drain_one()
                    # normalization epilogue for this q block
                    for h in range(HPC):
                        r = r_pool.tile([1, 512], f32, tag="r", name="r")
                        nc.vector.reciprocal_approx_fast(
                            r, ps2[32 * h : 32 * h + 1, :])
                        rbc = rbc_pool.tile([128, 512], f32, tag="rbc",
                                            name="rbc")
                        nc.gpsimd.partition_broadcast(rbc, r)
                        nc.vector.tensor_mul(ahat[b][:, h, jq], po[h], rbc)

            # ================ phase 0: batch-0 projections ================
            nc.enter_named_scope("proj0", False)
            run_all(qk_chains(0))
            run_all(v_chains(0))
            nc.leave_named_scope("proj0", None, False)

            # ====== phase 1: attn(b0) + proj(b1) interleave ===============
            nc.enter_named_scope("attn0", False)
            for c in range(4):
                nc.sync.dma_start(out=x8_sb[:, 2 * c : 2 * c + 2],
                                  in_=x8_in[1, :, 2 * c : 2 * c + 2])

            def proj1_gen():
                yield from qk_chains(1)
                yield from v_chains(1)

            attention(0, proj1_gen(), 128 + 256)
            nc.leave_named_scope("attn0", None, False)

            # ====== phase 2: attn(b1) + outproj(b0) interleave ============
            nc.enter_named_scope("attn1", False)
            attention(1, o_chains(0), 128)
            nc.leave_named_scope("attn1", None, False)

            # ================ phase 3: outproj(b1) tail ===================
            nc.enter_named_scope("tail", False)
            run_all(o_chains(1))
            nc.leave_named_scope("tail", 0, False)

    nc.compile()
    return nc


def _get_nc():
    if "nc" not in _COMPILED:
        _COMPILED["nc"] = _build_nc()
    return _COMPILED["nc"]


def _rope_tables():
    inv = (1.0 / (np.float32(10000.0)
                  ** (np.arange(0, HD, 2, dtype=np.float32) / np.float32(HD))))
    t = np.arange(N, dtype=np.float32)
    freqs = t[:, None] * inv[None, :].astype(np.float32)  # [N, HD/2]
    cosT = np.repeat(np.cos(freqs).astype(np.float32).T, 2, axis=0)  # [HD, N]
    s2T = np.repeat(np.sin(freqs).astype(np.float32).T, 2, axis=0).copy()
    s2T[0::2, :] *= np.float32(-1.0)
    return np.ascontiguousarray(cosT), np.ascontiguousarray(s2T)


def _make_in_maps(x, Wq, bq, Wk, bk, Wv, Wo):
    import ml_dtypes

    f8 = ml_dtypes.float8_e4m3fn
    cosT, s2T = _rope_tables()
    cosT = cosT.astype(np.float16)
    s2T = s2T.astype(np.float16)

    x = np.asarray(x, dtype=np.float32)
    xt = x.transpose(0, 2, 1)  # [B, D, N]
    # x8[b, p, c, g, n] = x[b, n, 128*(2c+g)+p]
    x8 = np.ascontiguousarray(
        xt.reshape(B, 8, 2, 128, N).transpose(0, 3, 1, 2, 4)).astype(f8)
    # xc[b, tt, p, dc, t] = x[b, tt*128+t, 128*dc+p]
    xc = np.ascontiguousarray(
        xt.reshape(B, DC, 128, NT, 128).transpose(0, 3, 2, 1, 4)
    ).astype(np.float16)

    in_maps = []
    for c in range(NCORES):
        cols = slice(c * DLOC, (c + 1) * DLOC)
        # w8[p, cc, g, m] = W[128*(2cc+g)+p, m]
        wq8 = np.ascontiguousarray(
            Wq[:, cols].reshape(8, 2, 128, DLOC).transpose(2, 0, 1, 3)
        ).astype(f8)
        wk8 = np.ascontiguousarray(
            Wk[:, cols].reshape(8, 2, 128, DLOC).transpose(2, 0, 1, 3)
        ).astype(f8)
        wv = np.ascontiguousarray(
            Wv[:, cols].reshape(DC, 128, DLOC).transpose(1, 0, 2)
        ).astype(np.float16)
        wo = np.ascontiguousarray(
            Wo[cols, :].reshape(HPC, 128, D).transpose(1, 0, 2)
        ).astype(np.float16)
        in_maps.append({
            "x8": x8,
            "xc": xc,
            "wq8": wq8,
            "wk8": wk8,
            "wv": wv,
            "wo": wo,
            "bq": np.ascontiguousarray(
                bq[cols].reshape(HPC, 128).T.astype(np.float32)),
            "bk": np.ascontiguousarray(
                bk[cols].reshape(HPC, 128).T.astype(np.float32)),
            "cosT": cosT,
            "s2T": s2T,
        })
    return in_maps


def run_device(x, Wq, bq, Wk, bk, Wv, bv, Wo, bo, trace=False):
    """Run the 8-core kernel; returns (full_output, BassKernelResults)."""
    from concourse.bass_utils import run_bass_kernel_spmd

    nc = _get_nc()
    in_maps = _make_in_maps(x, Wq, bq, Wk, bk, Wv, Wo)
    res = run_bass_kernel_spmd(nc, in_maps, core_ids=list(range(NCORES)),
                               trace=trace)
    acc = np.zeros((B, N, D), dtype=np.float64)
    for c in range(NCORES):
        acc += res.results[c]["out_p"]
    bias = (bv.astype(np.float64) @ Wo.astype(np.float64)
            + bo.astype(np.float64))
    out = (acc + bias).astype(np.float32)
    return out, res


def kernel(x, Wq, bq, Wk, bk, Wv, bv, Wo, bo):
    out, _ = run_device(x, Wq, bq, Wk, bk, Wv, bv, Wo, bo, trace=False)
    return out


# revision 9
# speedup vs baseline: 1.3104x; 1.1330x over previous
"""Multi-head attention with interleaved RoPE on 8 Trainium2 NeuronCores.

Tensor-parallel over heads (2 heads/core), restructured for continuous PE
occupancy (TRN2 PE drops to 1.2 GHz for 3us after any idle gap):

  - Q/K projections in fp8 e4m3 DoubleRow (K=256 per matmul, 2x PE rate).
    Only q/k are quantized; their error enters softmax logits where it is
    attenuated by the 1/d scaling, so the end-to-end error stays ~1e-2
    of absmax. V projection / attention / out-proj stay fp16.
  - Attention is software-pipelined (AV lags logits by 2 steps) so the PE
    never waits on the Scalar-engine Exp.
  - Projections for batch b+1 and the out-projection for batch b-1 are
    interleaved into the attention PE stream as fillers; the PE stream
    never has a dependency stall.
  - Softmax normalization is applied via a partition-broadcast of 1/s and
    one fused psum->sbuf multiply per (j, head).
  - Host folds softmax(.)@bv contribution (bv@Wo + bo) out of the device
    program and sums the 8 partial out-projections.

Layouts (per core):
  x8   [128, 8, 2, N] fp8   d-major, k-pair packed for DoubleRow
  xcol [NT][128, DC, 128]   tok-major fp16 columns for the V projection
  qT/kT[b] [128, HPC, N]    fp16, RoPE applied in place
  v[b] [128, NT, DLOC] fp16 (tok on partitions)
  ex   [128, 1024] fp16     exp(logits/d) for both heads of one key chunk
"""

import numpy as np

B = 2
N = 2048
D = 2048
H = 16
HD = 128
NCORES = 8
HPC = H // NCORES      # heads per core = 2
DLOC = HPC * HD        # local width = 256
DC = D // 128          # contraction chunks = 16
NT = N // 128          # token tiles = 16
NJ = N // 512          # 512-wide q blocks = 4

_COMPILED = {}


def _build_nc():
    import concourse.bacc as bacc
    import concourse.mybir as mybir
    import concourse.tile as tile

    f32 = mybir.dt.float32
    f16 = mybir.dt.float16
    f8 = mybir.dt.float8e4
    DR = mybir.MatmulPerfMode.DoubleRow
    Exp = mybir.ActivationFunctionType.Exp
    Ident = mybir.ActivationFunctionType.Identity
    inv_d = 1.0 / HD  # folds the module's two 1/sqrt(d) logit scalings

    nc = bacc.Bacc("TRN2", target_bir_lowering=False, debug=False,
                   num_devices=NCORES)

    x8_in = nc.dram_tensor("x8", [B, 128, 8, 2, N], f8,
                           kind="ExternalInput").ap()
    xc_in = nc.dram_tensor("xc", [B, NT, 128, DC, 128], f16,
                           kind="ExternalInput").ap()
    wq8_in = nc.dram_tensor("wq8", [128, 8, 2, DLOC], f8,
                            kind="ExternalInput").ap()
    wk8_in = nc.dram_tensor("wk8", [128, 8, 2, DLOC], f8,
                            kind="ExternalInput").ap()
    wv_in = nc.dram_tensor("wv", [128, DC, DLOC], f16,
                           kind="ExternalInput").ap()
    wo_in = nc.dram_tensor("wo", [128, HPC, D], f16,
                           kind="ExternalInput").ap()
    bq_in = nc.dram_tensor("bq", [128, HPC], f32, kind="ExternalInput").ap()
    bk_in = nc.dram_tensor("bk", [128, HPC], f32, kind="ExternalInput").ap()
    cos_in = nc.dram_tensor("cosT", [HD, N], f16, kind="ExternalInput").ap()
    s2_in = nc.dram_tensor("s2T", [HD, N], f16, kind="ExternalInput").ap()
    out_p = nc.dram_tensor("out_p", [B, N, D], f16, kind="ExternalOutput").ap()

    # swap even/odd partitions within each 32-lane quadrant (RoPE rotate)
    swap_mask = [i + 1 if i % 2 == 0 else i - 1 for i in range(32)]

    with tile.TileContext(nc) as tc:
        with (
            tc.tile_pool(name="persist", bufs=1) as pers,
            tc.tile_pool(name="pl", bufs=3, space="PSUM") as pl_pool,
            tc.tile_pool(name="po", bufs=1, space="PSUM") as po_pool,
            tc.tile_pool(name="ps2", bufs=1, space="PSUM") as ps2_pool,
            tc.tile_pool(name="pj", bufs=2, space="PSUM") as pj_pool,
            tc.tile_pool(name="pex", bufs=4) as ex_pool,
            tc.tile_pool(name="prp", bufs=8) as rope_pool,
            tc.tile_pool(name="pxc", bufs=4) as xcol_pool,
            tc.tile_pool(name="pob", bufs=4) as ob_pool,
            tc.tile_pool(name="prr", bufs=4) as r_pool,
            tc.tile_pool(name="prb", bufs=3) as rbc_pool,
        ):
            # ---- persistent SBUF tensors ---------------------------------
            wk8_sb = pers.tile([128, 8, 2, DLOC], f8, tag="wk8_sb")
            wq8_sb = pers.tile([128, 8, 2, DLOC], f8, tag="wq8_sb")
            x8_sb = pers.tile([128, 8, 2, N], f8, tag="x8_sb")
            wv_sb = pers.tile([128, DC, DLOC], f16, tag="wv_sb")
            wo_sb = pers.tile([128, HPC, D], f16, tag="wo_sb")
            cos_sb = pers.tile([HD, N], f16, tag="cos_sb")
            s2_sb = pers.tile([HD, N], f16, tag="s2_sb")
            bq_sb = pers.tile([128, HPC], f32, tag="bq_sb")
            bk_sb = pers.tile([128, HPC], f32, tag="bk_sb")
            zb = pers.tile([128, 1], f32, tag="zb")
            ones1 = pers.tile([128, 1], f16, tag="ones1")
            warm = pers.tile([128, 128], f16, tag="warm")

            qT = [pers.tile([128, HPC, N], f16, tag=f"qT{b}", name=f"qT{b}")
                  for b in range(B)]
            kT = [pers.tile([128, HPC, N], f16, tag=f"kT{b}", name=f"kT{b}")
                  for b in range(B)]
            v_sb = [pers.tile([128, NT, DLOC], f16, tag=f"v{b}", name=f"v{b}")
                    for b in range(B)]
            ahat = [pers.tile([128, HPC, N], f16, tag=f"ah{b}", name=f"ah{b}")
                    for b in range(B)]

            # initial loads: sync ring carries weights + x8(b0); gpsimd ring
            # carries the small tables.
            nc.sync.dma_start(out=wk8_sb, in_=wk8_in)
            nc.sync.dma_start(out=wq8_sb, in_=wq8_in)
            for c in range(4):
                nc.sync.dma_start(out=x8_sb[:, 2 * c : 2 * c + 2],
                                  in_=x8_in[0, :, 2 * c : 2 * c + 2])
            nc.sync.dma_start(out=wv_sb, in_=wv_in)
            nc.sync.dma_start(out=wo_sb, in_=wo_in)
            nc.gpsimd.dma_start(out=cos_sb, in_=cos_in)
            nc.gpsimd.dma_start(out=s2_sb, in_=s2_in)
            nc.gpsimd.dma_start(out=bq_sb, in_=bq_in)
            nc.gpsimd.dma_start(out=bk_sb, in_=bk_in)

            nc.vector.memset(zb, 0.0)
            nc.vector.memset(ones1, 1.0)
            nc.vector.memset(warm, 0.0)

            # warm the PE p-state while the input DMAs land
            for _ in range(100):
                pw = pj_pool.tile([128, 128], f32, tag="pj", name="pw")
                nc.tensor.matmul(pw, warm, warm, start=True, stop=True)

            # ---- filler generators --------------------------------------
            _cctr = [0]  # alternator for psum->sbuf copies
            _dctr = [0]  # alternator for output DMA rings

            def qk_chains(b):
                """fp8 DoubleRow Q/K projections + fused bias/RoPE epilogue.
                Yields one closure per PE matmul."""
                for w8sb, bcol, dst in ((wk8_sb, bk_sb, kT[b]),
                                        (wq8_sb, bq_sb, qT[b])):
                    for h in range(HPC):
                        for nch in range(NJ):
                            nsl = slice(nch * 512, (nch + 1) * 512)
                            pq = pj_pool.tile([128, 512], f32, tag="pj",
                                              name="pq")

                            def mk(c, pq=pq, w8sb=w8sb, h=h, nsl=nsl,
                                   bcol=bcol, dst=dst):
                                def emit():
                                    nc.tensor.matmul(
                                        pq,
                                        w8sb[:, c, :, h * 128 : (h + 1) * 128],
                                        x8_sb[:, c, :, nsl],
                                        start=(c == 0), stop=(c == 7),
                                        perf_mode=DR,
                                    )
                                    if c == 7:
                                        q1 = rope_pool.tile([128, 512], f16,
                                                            tag="rp", name="q1")
                                        nc.scalar.activation(
                                            q1, pq, Ident,
                                            bias=bcol[:, h : h + 1], scale=1.0)
                                        sw0 = rope_pool.tile([128, 512], f16,
                                                             tag="rp",
                                                             name="sw0")
                                        nc.vector.stream_shuffle(sw0, q1,
                                                                 swap_mask)
                                        tm = rope_pool.tile([128, 512], f16,
                                                            tag="rp", name="tm")
                                        nc.vector.tensor_mul(
                                            tm, q1, cos_sb[:, nsl])
                                        sw1 = rope_pool.tile([128, 512], f16,
                                                             tag="rp",
                                                             name="sw1")
                                        nc.gpsimd.tensor_mul(
                                            sw1, sw0, s2_sb[:, nsl])
                                        nc.vector.tensor_add(
                                            dst[:, h, nsl], tm, sw1)
                                return emit

                            for c in range(8):
                                yield mk(c)

            def v_chains(b):
                """fp16 V projection from streamed token-column tiles."""
                for tt in range(NT):
                    xc = xcol_pool.tile([128, DC, 128], f16, tag="xc",
                                        name="xc")
                    pv = pj_pool.tile([128, DLOC], f32, tag="pj", name="pv")

                    def mk(dc, xc=xc, pv=pv, tt=tt, b=b):
                        def emit():
                            if dc == 0:
                                nc.sync.dma_start(out=xc, in_=xc_in[b, tt])
                            nc.tensor.matmul(
                                pv, xc[:, dc, :], wv_sb[:, dc, :],
                                start=(dc == 0), stop=(dc == DC - 1))
                            if dc == DC - 1:
                                nc.scalar.copy(v_sb[b][:, tt, :], pv)
                        return emit

                    for dc in range(DC):
                        yield mk(dc)

            def o_chains(b, dve_only=False):
                """fp16 out-projection of normalized attention output.
                dve_only: route psum->sbuf copies off the scalar engine
                (used while Exp saturates it)."""
                for tt in range(NT):
                    tsl = slice(tt * 128, (tt + 1) * 128)
                    for n in range(4):
                        nsl = slice(n * 512, (n + 1) * 512)
                        pp = pj_pool.tile([128, 512], f32, tag="pj", name="pp")

                        def mk(h, pp=pp, tsl=tsl, nsl=nsl, b=b):
                            def emit():
                                nc.tensor.matmul(
                                    pp, ahat[b][:, h, tsl],
                                    wo_sb[:, h, nsl],
                                    start=(h == 0), stop=(h == HPC - 1))
                                if h == HPC - 1:
                                    ob = ob_pool.tile([128, 512], f16,
                                                      tag="ob", name="ob")
                                    if dve_only or _cctr[0] % 2:
                                        nc.vector.tensor_copy(ob, pp)
                                    else:
                                        nc.scalar.copy(ob, pp)
                                    _cctr[0] += 1
                                    eng = (nc.sync if _dctr[0] % 2 == 0
                                           else nc.gpsimd)
                                    _dctr[0] += 1
                                    eng.dma_start(out=out_p[b, tsl, nsl],
                                                  in_=ob)
                            return emit

                        for h in range(HPC):
                            yield mk(h)

            def run_all(gen):
                for emit in gen:
                    emit()

            # ---- attention with pipelined drain + fillers ----------------
            def attention(b, fillers, n_fill):
                """64 steps of (2 logits mm, 2 Exp, lagged 2 AV + 2 sum mms),
                popping fillers to keep the PE stream dense."""
                popped = [0]
                step = [0]
                n_calls = NJ * (NT + 2)  # i-steps plus per-j tail drains

                def pop_fillers():
                    step[0] += 1
                    want = (n_fill * step[0]) // n_calls
                    while popped[0] < want:
                        emit = next(fillers, None)
                        if emit is None:
                            popped[0] = n_fill
                            return
                        emit()
                        popped[0] += 1

                for j in range(NJ):
                    jq = slice(j * 512, (j + 1) * 512)
                    po = [po_pool.tile([128, 512], f32, tag=f"po{h}",
                                       name=f"po{h}") for h in range(HPC)]
                    ps2 = ps2_pool.tile([64, 512], f32, tag="ps2", name="ps2")
                    exq = []

                    def drain_one():
                        i2, exs = exq.pop(0)
                        for h in range(HPC):
                            nc.tensor.matmul(
                                po[h], v_sb[b][:, i2, h * 128 : (h + 1) * 128],
                                exs[h], start=(i2 == 0), stop=(i2 == NT - 1))
                            nc.tensor.matmul(
                                ps2[32 * h : 32 * h + 1, :], ones1, exs[h],
                                start=(i2 == 0), stop=(i2 == NT - 1))

                    for i in range(NT):
                        exs = []
                        for h in range(HPC):
                            pl = pl_pool.tile([128, 512], f32, tag="pl",
                                              name="pl")
                            nc.tensor.matmul(
                                pl,
                                kT[b][:, h, i * 128 : (i + 1) * 128],
                                qT[b][:, h, jq],
                                start=True, stop=True)
                            ex = ex_pool.tile([128, 512], f16, tag="ex",
                                              name="ex")
                            nc.scalar.activation(ex, pl, Exp, bias=zb,
                                                 scale=inv_d)
                            exs.append(ex)
                        exq.append((i, exs))
                        if len(exq) > 2:
                            drain_one()
                        pop_fillers()
                    while exq:
                        drain_one()
                        pop_fillers()
                    # normalization epilogue for this q block
                    for h in range(HPC):
                        r = r_pool.tile([1, 512], f32, tag="r", name="r")
                        nc.vector.reciprocal_approx_fast(
                            r, ps2[32 * h : 32 * h + 1, :])
                        rbc = rbc_pool.tile([128, 512], f32, tag="rbc",
                                            name="rbc")
                        nc.gpsimd.partition_broadcast(rbc, r)
                        nc.vector.tensor_mul(ahat[b][:, h, jq], po[h], rbc)

            # ================ phase 0: batch-0 projections ================
            nc.enter_named_scope("proj0", False)
            run_all(qk_chains(0))
            # x8(b1) load on the scalar ring (sync carries the xcol stream):
            # the ring instruction waits on the last b0 QK-proj read, which
            # completes early in the V-projection, so the reload streams in
            # well ahead of the b1 QK filler chains.
            for c in range(4):
                nc.scalar.dma_start(out=x8_sb[:, 2 * c : 2 * c + 2],
                                    in_=x8_in[1, :, 2 * c : 2 * c + 2])
            run_all(v_chains(0))
            nc.leave_named_scope("proj0", None, False)

            # ====== phase 1: attn(b0) + proj(b1) interleave ===============
            nc.enter_named_scope("attn0", False)

            def proj1_gen():
                yield from qk_chains(1)
                yield from v_chains(1)

            attention(0, proj1_gen(), 128 + 256)
            nc.leave_named_scope("attn0", None, False)

            # ====== phase 2: attn(b1) + outproj(b0) interleave ============
            nc.enter_named_scope("attn1", False)
            attention(1, o_chains(0, dve_only=True), 128)
            nc.leave_named_scope("attn1", None, False)

            # ================ phase 3: outproj(b1) tail ===================
            nc.enter_named_scope("tail", False)
            run_all(o_chains(1))
            nc.leave_named_scope("tail", 0, False)

    nc.compile()
    return nc


def _get_nc():
    if "nc" not in _COMPILED:
        _COMPILED["nc"] = _build_nc()
    return _COMPILED["nc"]


def _rope_tables():
    inv = (1.0 / (np.float32(10000.0)
                  ** (np.arange(0, HD, 2, dtype=np.float32) / np.float32(HD))))
    t = np.arange(N, dtype=np.float32)
    freqs = t[:, None] * inv[None, :].astype(np.float32)  # [N, HD/2]
    cosT = np.repeat(np.cos(freqs).astype(np.float32).T, 2, axis=0)  # [HD, N]
    s2T = np.repeat(np.sin(freqs).astype(np.float32).T, 2, axis=0).copy()
    s2T[0::2, :] *= np.float32(-1.0)
    return np.ascontiguousarray(cosT), np.ascontiguousarray(s2T)


def _make_in_maps(x, Wq, bq, Wk, bk, Wv, Wo):
    import ml_dtypes

    f8 = ml_dtypes.float8_e4m3fn
    cosT, s2T = _rope_tables()
    cosT = cosT.astype(np.float16)
    s2T = s2T.astype(np.float16)

    x = np.asarray(x, dtype=np.float32)
    xt = x.transpose(0, 2, 1)  # [B, D, N]
    # x8[b, p, c, g, n] = x[b, n, 128*(2c+g)+p]
    x8 = np.ascontiguousarray(
        xt.reshape(B, 8, 2, 128, N).transpose(0, 3, 1, 2, 4)).astype(f8)
    # xc[b, tt, p, dc, t] = x[b, tt*128+t, 128*dc+p]
    xc = np.ascontiguousarray(
        xt.reshape(B, DC, 128, NT, 128).transpose(0, 3, 2, 1, 4)
    ).astype(np.float16)

    in_maps = []
    for c in range(NCORES):
        cols = slice(c * DLOC, (c + 1) * DLOC)
        # w8[p, cc, g, m] = W[128*(2cc+g)+p, m]
        wq8 = np.ascontiguousarray(
            Wq[:, cols].reshape(8, 2, 128, DLOC).transpose(2, 0, 1, 3)
        ).astype(f8)
        wk8 = np.ascontiguousarray(
            Wk[:, cols].reshape(8, 2, 128, DLOC).transpose(2, 0, 1, 3)
        ).astype(f8)
        wv = np.ascontiguousarray(
            Wv[:, cols].reshape(DC, 128, DLOC).transpose(1, 0, 2)
        ).astype(np.float16)
        wo = np.ascontiguousarray(
            Wo[cols, :].reshape(HPC, 128, D).transpose(1, 0, 2)
        ).astype(np.float16)
        in_maps.append({
            "x8": x8,
            "xc": xc,
            "wq8": wq8,
            "wk8": wk8,
            "wv": wv,
            "wo": wo,
            "bq": np.ascontiguousarray(
                bq[cols].reshape(HPC, 128).T.astype(np.float32)),
            "bk": np.ascontiguousarray(
                bk[cols].reshape(HPC, 128).T.astype(np.float32)),
            "cosT": cosT,
            "s2T": s2T,
        })
    return in_maps


def run_device(x, Wq, bq, Wk, bk, Wv, bv, Wo, bo, trace=False):
    """Run the 8-core kernel; returns (full_output, BassKernelResults)."""
    from concourse.bass_utils import run_bass_kernel_spmd

    nc = _get_nc()
    in_maps = _make_in_maps(x, Wq, bq, Wk, bk, Wv, Wo)
    res = run_bass_kernel_spmd(nc, in_maps, core_ids=list(range(NCORES)),
                               trace=trace)
    acc = np.zeros((B, N, D), dtype=np.float64)
    for c in range(NCORES):
        acc += res.results[c]["out_p"]
    bias = (bv.astype(np.float64) @ Wo.astype(np.float64)
            + bo.astype(np.float64))
    out = (acc + bias).astype(np.float32)
    return out, res


def kernel(x, Wq, bq, Wk, bk, Wv, bv, Wo, bo):
    out, _ = run_device(x, Wq, bq, Wk, bk, Wv, bv, Wo, bo, trace=False)
    return out
